# revision 10
# baseline (speedup 1.0000x reference)
"""Trainium2 Bass kernel for nn_CorrLoss — v5.

fp8e4 DoubleRow Gram matmuls (3 k-pair steps per [128,512] tile) with the
one-hot class-code contraction extension (corr' = corr - 16384*same).
Rows+columns class-sorted on host: the min-over-positives side is a few
narrow exact f32 PSUM window reduces; the max-over-negatives side is a
running elementwise max: per round, tile m3 folds straight from PSUM f32
on DVE while tiles m0-2 are staged to SBUF as bf16 by the Act engine and
folded at 2x DVE rate.  One [128,4,512] reduce finishes the max side.
Row-data-parallel across 8 cores via rotated column-chunk upload.
"""
import sys
from contextlib import ExitStack

import numpy as np

sys.path.insert(0, "/opt/trn_rl_repo")

import concourse.bass as bass  # noqa: E402
from concourse import mybir  # noqa: E402
from concourse.bass_utils import run_bass_kernel_spmd  # noqa: E402

N_CORES = 8
N_ROWS = 4096
D = 512
NCLS = 64
M = N_ROWS // N_CORES      # 512 local rows
MT = M // 128              # 4 row tiles
NCHUNK = 512
NT = N_ROWS // NCHUNK      # 8 column chunks
KE = 6                     # 4 feature k-blocks + one-hot ext + zero pad
KP = KE // 2               # 3 DoubleRow pairs
SLOTW = KE * NCHUNK        # 3072
MARGIN = 40.0
BIG = 16384.0
CODE = 128.0
N_WARMUP = 48
W_WARMUP = 64
L = [1, 0, 2, 3, 4, 5, 6, 7]
POS_OF_SLOT = {1: 0, 0: 1, 2: 2}

_CACHE = {}


def _window_pieces(mar):
    pieces = []
    for m in range(MT):
        lo = 512 + 128 * m - mar
        hi = 512 + 128 * m + 128 + mar
        for s in (0, 1, 2):
            a = max(lo, 512 * s)
            b = min(hi, 512 * s + 512)
            if a < b:
                c = 4 * POS_OF_SLOT[s] + m
                pieces.append((m, c, a - 512 * s, b - 512 * s))
    pieces.sort(key=lambda p: p[1])
    return pieces


def _build(mar):
    f32 = mybir.dt.float32
    bf16 = mybir.dt.bfloat16
    fp8 = mybir.dt.float8e4
    op = mybir.AluOpType
    ax = mybir.AxisListType
    DR = mybir.MatmulPerfMode.DoubleRow
    FMAX = 3.4e38
    pieces = _window_pieces(mar)
    n_pieces = len(pieces)

    def wsem_need(r):
        return sum(1 for (_m, c, _a, _b) in pieces if c < 4 * (r - 1))

    nc = bass.Bass("TRN2", target_bir_lowering=False, debug=False)
    # host layout: [lx(ext,pad: 1024) | region0 (3072) | ... | region7]
    rx = nc.declare_dram_parameter("rx", [128, 2 * NCHUNK + NT * SLOTW], fp8,
                                   isOutput=False)
    pl = nc.declare_dram_parameter("pl", [128, 2 * MT], f32, isOutput=True)

    def bank(c):
        return ((c // 4) % 2) * 4 + (c % 4)

    with ExitStack() as ctx:
        # dim1 k-blocks: 0-1 = lx (+code ext, zero pad); region r at 2+6r
        rx3 = ctx.enter_context(
            nc.sbuf_tensor("rx3", [128, 2 + NT * KE, NCHUNK], fp8))
        wup = ctx.enter_context(nc.sbuf_tensor("wup", [128, 128], fp8))
        cbuf = [ctx.enter_context(nc.sbuf_tensor(f"cbuf{i}", [128, 3, NCHUNK], bf16))
                for i in range(2)]
        rmx = ctx.enter_context(nc.sbuf_tensor("rmx", [128, MT, NCHUNK], bf16))
        rw = ctx.enter_context(nc.sbuf_tensor("rw", [128, n_pieces], f32))
        an6 = ctx.enter_context(nc.sbuf_tensor("an6", [128, MT], f32))
        apv = ctx.enter_context(nc.sbuf_tensor("apv", [128, MT], f32))
        anv = ctx.enter_context(nc.sbuf_tensor("anv", [128, MT], f32))
        pl_sb = ctx.enter_context(nc.sbuf_tensor("pl_sb", [128, 2 * MT], f32))
        ptall = ctx.enter_context(nc.psum_tensor("ptall", [128, 8, NCHUNK], f32))
        dsem = [ctx.enter_context(nc.semaphore(f"dsem{r}")) for r in range(NT)]
        dout = ctx.enter_context(nc.semaphore("dout"))
        mm_sem = ctx.enter_context(nc.semaphore("mm_sem"))
        asem = ctx.enter_context(nc.semaphore("asem"))
        fsem = ctx.enter_context(nc.semaphore("fsem"))
        wsem = ctx.enter_context(nc.semaphore("wsem"))
        isem = ctx.enter_context(nc.semaphore("isem"))
        done_sem = ctx.enter_context(nc.semaphore("done_sem"))
        block = ctx.enter_context(nc.Block())

        @block.gpsimd
        def _(gpsimd):
            nc.gpsimd.memset(wup[:], 0.0).then_inc(isem, 1)
            nc.gpsimd.memset(rmx[:], -FMAX).then_inc(isem, 1)

        @block.sync
        def _(sync):
            # region0 first (kp0/kp1 of round 0), lx second (kp2)
            sync.dma_start(rx3[:, 2:2 + KE, :],
                           rx[:, 0:KE * NCHUNK]).then_inc(dsem[0], 16)
            sync.dma_start(rx3[:, 0:2, :],
                           rx[:, KE * NCHUNK:(2 + KE) * NCHUNK]).then_inc(
                               dsem[0], 16)
            for r in range(1, NT):
                o = 2 * NCHUNK + r * SLOTW
                sync.dma_start(rx3[:, 2 + r * KE:2 + (r + 1) * KE, :],
                               rx[:, o:o + SLOTW]).then_inc(dsem[r], 16)
            sync.wait_ge(done_sem, 1)
            sync.dma_start(pl[:], pl_sb[:]).then_inc(dout, 16)
            sync.wait_ge(dout, 16)

        @block.tensor
        def _(tensor):
            tensor.wait_ge(isem, 1)
            for _ in range(N_WARMUP):
                nc.tensor.matmul(ptall[:, 7, 0:W_WARMUP], wup[:],
                                 wup[:, 0:W_WARMUP], start=True, stop=True)
            for r in range(NT):
                tensor.wait_ge(dsem[r], 16)
                for m in range(MT):
                    c = r * MT + m
                    b = bank(c)
                    if r >= 2:
                        # this tile's bank consumer from round r-2: the Act
                        # copy (m0-2) or the DVE direct fold (m3), plus any
                        # min-window reads of that tile
                        if m == 0:
                            tensor.wait_ge(asem, r - 1)
                        if m == MT - 1:
                            tensor.wait_ge(fsem, 2 * r - 3)
                        w = sum(1 for (_pm, pc, _a, _b) in pieces
                                if pc <= 4 * (r - 2) + m)
                        if w > 0:
                            tensor.wait_ge(wsem, w)
                    out = ptall[:, b, :]
                    ro = 2 + r * KE
                    for kp in range(KP - 1):
                        nc.tensor.matmul(
                            out,
                            rx3[:, 2 + 2 * kp:2 + 2 * kp + 2,
                                m * 128:(m + 1) * 128],
                            rx3[:, ro + 2 * kp:ro + 2 * kp + 2, :],
                            perf_mode=DR, start=(kp == 0), stop=False)
                    if r == 0 and m == 0:
                        tensor.wait_ge(dsem[0], 32)
                    mm = nc.tensor.matmul(
                        out,
                        rx3[:, 0:2, m * 128:(m + 1) * 128],
                        rx3[:, ro + 4:ro + 6, :],
                        perf_mode=DR, start=False, stop=True)
                    mm.then_inc(mm_sem, 1)

        @block.scalar
        def _(scalar):
            # stage tiles m0-2 of each round into SBUF bf16 for the DVE fold
            for r in range(NT):
                scalar.wait_ge(mm_sem, 4 * r + 3)
                if r >= 2:
                    scalar.wait_ge(fsem, 2 * (r - 2) + 2)
                h = (r % 2) * 4
                nc.scalar.copy(cbuf[r % 2][:],
                               ptall[:, h:h + 3, :]).then_inc(asem, 1)

        @block.vector
        def _(vector):
            vector.wait_ge(isem, 2)
            def emit_piece(i):
                m, c, a, b = pieces[i]
                vector.wait_ge(mm_sem, c + 1)
                bk = bank(c)
                nc.vector.tensor_reduce(
                    rw[:, i:i + 1], ptall[:, bk, a:b],
                    axis=ax.X, op=op.min).then_inc(wsem, 1)
            i = 0
            while i < n_pieces and pieces[i][1] < 8:
                emit_piece(i)
                i += 1
            for r in range(NT - 1):
                while i < n_pieces and pieces[i][1] + 1 <= 4 * r + 4:
                    emit_piece(i)
                    i += 1
                # direct f32 fold of tile m3 (gated only on the PE)
                vector.wait_ge(mm_sem, 4 * r + 4)
                h = (r % 2) * 4
                nc.vector.tensor_tensor(
                    rmx[:, MT - 1, :], ptall[:, h + 3, :],
                    rmx[:, MT - 1, :], op=op.max).then_inc(fsem, 1)
                # bf16 fold of the staged m0-2 tiles (2x DVE rate)
                vector.wait_ge(asem, r + 1)
                nc.vector.tensor_tensor(
                    rmx[:, 0:MT - 1, :], cbuf[r % 2][:], rmx[:, 0:MT - 1, :],
                    op=op.max).then_inc(fsem, 1)
                if r == 3:
                    # min-side combines: all window pieces are in by round 3
                    # and DVE has slack here, keeping them off the tail
                    nc.vector.drain()
                    for m in range(MT):
                        idx = [j for j, p in enumerate(pieces) if p[0] == m]
                        if len(idx) == 1:
                            nc.vector.tensor_scalar(
                                pl_sb[:, MT + m:MT + m + 1],
                                rw[:, idx[0]:idx[0] + 1], FMAX, 0.0,
                                op0=op.min)
                        else:
                            nc.vector.tensor_tensor(
                                pl_sb[:, MT + m:MT + m + 1],
                                rw[:, idx[0]:idx[0] + 1],
                                rw[:, idx[1]:idx[1] + 1], op=op.min)
            # round 7: fold m3 directly, fold the staged m0-2, then one
            # reduce over the whole running max
            rl = NT - 1
            vector.wait_ge(mm_sem, 4 * rl + 4)
            h = (rl % 2) * 4
            nc.vector.tensor_tensor(
                rmx[:, MT - 1, :], ptall[:, h + 3, :],
                rmx[:, MT - 1, :], op=op.max)
            vector.wait_ge(asem, rl + 1)
            nc.vector.tensor_tensor(
                rmx[:, 0:MT - 1, :], cbuf[rl % 2][:], rmx[:, 0:MT - 1, :],
                op=op.max)
            fin = nc.vector.tensor_reduce(
                pl_sb[:, 0:MT], rmx[:], axis=ax.X, op=op.max)
            fin.then_inc(done_sem, 1)
    return nc


def _prep(feat: np.ndarray, targets: np.ndarray):
    import ml_dtypes
    fp8 = ml_dtypes.float8_e4m3
    feat = np.asarray(feat, dtype=np.float32)
    tg = np.asarray(targets).astype(np.int64)

    perm = np.argsort(tg, kind="stable")
    tgs = tg[perm]
    feats = feat[perm]

    counts = np.bincount(tgs, minlength=NCLS)
    # window margin: a block's first/last class can extend at most
    # count-1 columns beyond the block boundary
    mar = max(32, int(counts.max()) - 1)
    assert mar <= 384, "class sizes too skewed for the window scheme"

    featx = np.zeros((KE * 128, N_ROWS), dtype=fp8)
    featx[:D, :] = feats.T.astype(fp8)
    featx[D + tgs, np.arange(N_ROWS)] = fp8(-CODE)

    in_maps = []
    for c in range(N_CORES):
        rxa = np.empty((128, 2 * NCHUNK + NT * SLOTW), dtype=fp8)
        tloc = tgs[c * M:(c + 1) * M]
        lxa = np.zeros((128, 2 * NCHUNK), dtype=fp8)
        lxa[tloc, np.arange(M)] = fp8(CODE)
        # layout: [region0 | lx | region1..7] (region0 DMAs first)
        rxa[:, KE * NCHUNK:(2 + KE) * NCHUNK] = lxa
        for r in range(NT):
            gc = (c - 1 + L[r]) % NT
            blk = featx[:, gc * NCHUNK:(gc + 1) * NCHUNK]
            dst = 0 if r == 0 else 2 * NCHUNK + r * SLOTW
            rxa[:, dst:dst + SLOTW] = (
                blk.reshape(KE, 128, NCHUNK).transpose(1, 0, 2).reshape(128, SLOTW))
        in_maps.append({"rx": rxa})
    return in_maps, mar


def kernel(feat: np.ndarray, targets: np.ndarray) -> np.ndarray:
    in_maps, mar = _prep(feat, targets)
    key = ("nc", mar)
    if key not in _CACHE:
        _CACHE[key] = _build(mar)
    nc = _CACHE[key]
    # rare transient HW glitches surface as inf/NaN in the outputs; the
    # result is cheap to validate, so retry a couple of times if detected
    for _attempt in range(3):
        res = run_bass_kernel_spmd(nc, in_maps, list(range(N_CORES)))
        total = 0.0
        ok = True
        for c in range(N_CORES):
            out = res.results[c]["pl"].astype(np.float64)
            if not np.isfinite(out).all():
                ok = False
                break
            an = out[:, :MT]
            ap = out[:, MT:]
            total += np.maximum(an - ap + MARGIN - BIG, 0.0).sum()
        loss = total / N_ROWS
        if ok and np.isfinite(loss) and 0.0 <= loss <= 4.0e4:
            break
    return np.asarray(np.float32(loss))
